# revision 3
# baseline (speedup 1.0000x reference)
"""GCN 2-layer TRN2 kernel v3 — d=2 ap_gather (4 streams) + one-hot matmul.

Vs v2: table stores feature-PAIRS per partition ([128, W, 2]; partition
32s+p = feats (2p, 2p+1) of stream s's window), so each gather index moves a
full 64-feat message per stream and 4 streams run per instruction (4x fewer
gather idxs). Streams = (window of pair) x (dst half). Chunks are 64 slots;
PE transpose of the (k-major) [32, 128] view yields [128, 32] with rows
(k*64+s); two K=64 matmuls per chunk (even/odd feat planes) accumulate per
dst-tile in PSUM within a gather sub-batch; drains add into acc [128, 98, 64]
with stride-2 feature interleave.
"""

import numpy as np


class Cfg:
    def __init__(self):
        self.N = 100000
        self.E = 1200000
        self.ncores = 8
        self.shard = 12500
        self.ntile = 98
        self.tilecols = 12544              # table cols per window
        self.rounds = 4
        self.nstream = 4
        self.htile = 49                    # tiles per dst-half
        self.chunk = 128                   # slots per chunk
        self.din, self.dh, self.dout = 128, 64, 40
        self.subslots = 2048               # slots per gather (32 chunks)


CFG = Cfg()


class Sched:
    __slots__ = ("ct", "nchunk", "S", "Sused", "nsub", "gidx", "dstcol",
                 "wgt", "chunk_tile", "chunk_first", "chunk_last", "coff")


def build_sched(edge_index, cfg: Cfg):
    src = np.asarray(edge_index[0], dtype=np.int64)
    dst = np.asarray(edge_index[1], dtype=np.int64)
    N, shard, ncores = cfg.N, cfg.shard, cfg.ncores
    CH, SUB = cfg.chunk, cfg.subslots

    deg = np.bincount(dst, minlength=N).astype(np.int64) + 1
    dinv = (1.0 / np.sqrt(deg.astype(np.float64))).astype(np.float32)

    c_of = dst // shard
    w_of = src // shard
    order = np.lexsort((dst, c_of, w_of))
    src_s, dst_s = src[order], dst[order]
    wc_key = w_of[order] * ncores + c_of[order]
    bounds = np.searchsorted(wc_key, np.arange(8 * ncores + 1))

    scheds = []
    for r in range(cfg.rounds):
        sc = Sched()
        # split edges: stream s = 2*h + q (h: window in pair, q: dst half)
        cnt = np.zeros((cfg.nstream, cfg.htile, ncores), np.int64)
        per = {}
        for h in (0, 1):
            w = 2 * r + h
            for c in range(ncores):
                lo, hi = bounds[w * ncores + c], bounds[w * ncores + c + 1]
                s_ = src_s[lo:hi] - w * shard
                d_ = dst_s[lo:hi] - c * shard
                tl = d_ // 128
                for q in (0, 1):
                    m = (tl >= q * cfg.htile) & (tl < (q + 1) * cfg.htile)
                    st = 2 * h + q
                    np.add.at(cnt, (st, tl[m] - q * cfg.htile, c), 1)
                    per[(st, c)] = (s_[m], d_[m], tl[m]) if (st, c) not in per \
                        else per[(st, c)]
                    per[(st, c)] = (s_[m], d_[m], tl[m])
        # chunks per (stream, tile-in-half)
        ct = (cnt.max(axis=2) + CH - 1) // CH        # [nstream, htile]
        nchunk = ct.sum(axis=1)                      # per stream
        S_s = nchunk * CH
        S = int(((S_s.max() + SUB - 1) // SUB) * SUB)
        sc.ct = ct
        sc.nchunk = nchunk
        sc.S = S
        sc.Sused = int(S_s.max())
        sc.nsub = (sc.Sused + SUB - 1) // SUB

        # per-stream chunk->tile map and first/last flags (incl sub-batch cuts)
        sc.chunk_tile, sc.chunk_first, sc.chunk_last, sc.coff = [], [], [], []
        coff = 0
        for st in range(cfg.nstream):
            q = st % 2
            tiles = np.repeat(np.arange(cfg.htile) + q * cfg.htile, ct[st])
            nc_ = len(tiles)
            first = np.zeros(nc_, bool)
            last = np.zeros(nc_, bool)
            for j in range(nc_):
                first[j] = (j == 0 or tiles[j - 1] != tiles[j]
                            or (j % (SUB // CH)) == 0)
                last[j] = (j == nc_ - 1 or tiles[j + 1] != tiles[j]
                           or ((j + 1) % (SUB // CH)) == 0)
            sc.chunk_tile.append(tiles)
            sc.chunk_first.append(first)
            sc.chunk_last.append(last)
            sc.coff.append(coff)
            coff += nc_
        nctot = coff

        # per-core tensors
        sc.gidx, sc.dstcol, sc.wgt = [], [], []
        for c in range(ncores):
            gidx = np.zeros((128, S // 16), np.int16)
            dcol = np.zeros((128, nctot), np.float32)
            wcol = np.zeros((128, nctot), np.float32)
            for st in range(cfg.nstream):
                s_, d_, tl = per[(st, c)]
                q = st % 2
                # slot positions: per tile, rank within tile
                tlh = tl - q * cfg.htile
                o2 = np.argsort(tlh, kind="stable")
                s_, d_, tlh = s_[o2], d_[o2], tlh[o2]
                tile_base = np.zeros(cfg.htile + 1, np.int64)
                tile_base[1:] = np.cumsum(ct[st]) * CH
                starts = np.searchsorted(tlh, np.arange(cfg.htile))
                rank = np.arange(len(tlh)) - starts[tlh]
                pos = tile_base[tlh] + rank
                gi = np.zeros(S, np.int64)
                dc = np.zeros(S, np.float32)
                wg = np.zeros(S, np.float32)
                gi[pos] = s_
                dc[pos] = (d_ % 128).astype(np.float32)
                wg[pos] = dinv[s_ + (2 * r + st // 2) * shard] * \
                    dinv[d_ + c * shard]
                # wrap into the stream's two 16-part groups (dup)
                w16 = np.ascontiguousarray(gi.reshape(-1, 16).T.astype(np.int16))
                gidx[32 * st:32 * st + 16] = w16
                gidx[32 * st + 16:32 * st + 32] = w16
                # chunk columns: partitions j and 64+j = slot j of chunk
                nc_ = sc.nchunk[st]
                dcc = dc[:nc_ * CH].reshape(nc_, CH).T      # [128, nchunk]
                wgc = wg[:nc_ * CH].reshape(nc_, CH).T
                o = sc.coff[st]
                dcol[:, o:o + nc_] = dcc
                wcol[:, o:o + nc_] = wgc
            sc.gidx.append(np.ascontiguousarray(gidx))
            sc.dstcol.append(np.ascontiguousarray(dcol))
            sc.wgt.append(np.ascontiguousarray(wcol))
        scheds.append(sc)
    return scheds, dinv


# ---------------------------------------------------------------- golden

def golden(inputs, cfg: Cfg = CFG):
    x = np.asarray(inputs["x"], np.float32)
    ei = np.asarray(inputs["edge_index"])
    W1 = np.asarray(inputs["W1"], np.float32)
    b1 = np.asarray(inputs["b1"], np.float32)
    W2 = np.asarray(inputs["W2"], np.float32)
    b2 = np.asarray(inputs["b2"], np.float32)
    scheds, dinv = build_sched(ei, cfg)

    def propagate(tabT):
        acc = (tabT * (dinv ** 2)[:, None]).astype(np.float32)
        for c in range(cfg.ncores):
            for r, sc in enumerate(scheds):
                for st in range(cfg.nstream):
                    w = 2 * r + st // 2
                    gi = sc.gidx[c][32 * st:32 * st + 16]
                    gi = gi.T.reshape(-1).astype(np.int64)     # [S]
                    msg = tabT[gi + w * cfg.shard]             # [S, 64]
                    o = sc.coff[st]
                    for j in range(sc.nchunk[st]):
                        t = sc.chunk_tile[st][j]
                        seg = msg[j * 128:(j + 1) * 128]       # [128, 64f]
                        dc = sc.dstcol[c][:, o + j].astype(np.int64)
                        wg = sc.wgt[c][:, o + j]
                        oh = np.zeros((128, 128), np.float32)
                        oh[np.arange(128), dc] = wg
                        nj = min(128, cfg.shard - t * 128)
                        acc[c * cfg.shard + t * 128:
                            c * cfg.shard + t * 128 + nj] += (oh.T @ seg)[:nj]
        return acc

    h1 = x @ W1
    acc1 = propagate(h1)
    hid = acc1 + b1
    hid = np.where(hid > 0, hid, 0.01 * hid)
    acc2 = propagate(hid)
    return acc2 @ W2 + b2


# ---------------------------------------------------------------- bass

def build_bass(scheds, cfg: Cfg, reps: int = 1):
    import concourse.bass as bass
    import concourse.mybir as mybir
    import concourse.tile as tile
    from concourse import bacc
    from concourse.masks import make_identity

    f32 = mybir.dt.float32
    i16 = mybir.dt.int16
    P = 128
    dh, dout, shard, ntile = cfg.dh, cfg.dout, cfg.shard, cfg.ntile
    TC = cfg.tilecols
    CH, SUB = cfg.chunk, cfg.subslots

    nc = bacc.Bacc(None, target_bir_lowering=False)

    xT = nc.declare_dram_parameter("xT", [cfg.din, shard], f32, isOutput=False)
    W1p = nc.declare_dram_parameter("W1", [cfg.din, dh], f32, isOutput=False)
    b1p = nc.declare_dram_parameter("b1", [1, dh], f32, isOutput=False)
    W2p = nc.declare_dram_parameter("W2", [dh, dout], f32, isOutput=False)
    b2p = nc.declare_dram_parameter("b2", [1, dout], f32, isOutput=False)
    d2p = nc.declare_dram_parameter("dinv2", [P, ntile], f32, isOutput=False)
    iotap = nc.declare_dram_parameter("iota", [P, P], f32, isOutput=False)
    selp = nc.declare_dram_parameter("sel", [dh, dh], f32, isOutput=False)
    gidxp = [nc.declare_dram_parameter(f"gidx{r}", [P, scheds[r].S // 16], i16,
                                       isOutput=False) for r in range(4)]
    nct = [int(scheds[r].nchunk.sum()) for r in range(4)]
    dcp = [nc.declare_dram_parameter(f"dstcol{r}", [P, nct[r]], f32,
                                     isOutput=False) for r in range(4)]
    wgp = [nc.declare_dram_parameter(f"wgt{r}", [P, nct[r]], f32,
                                     isOutput=False) for r in range(4)]
    outp = nc.declare_dram_parameter("out", [shard, dout], f32, isOutput=True)

    # interleaved half-feature tables: row p of window w = feats (2p, 2p+1)
    ag_in = [nc.dram_tensor(f"ag_in{l}", [32, TC * 2], f32) for l in (0, 1)]
    tabd = [nc.dram_tensor(f"table{l}", [cfg.ncores * 32, TC * 2], f32,
                           addr_space="Shared") for l in (0, 1)]

    core_ids = list(range(cfg.ncores))

    with tile.TileContext(nc) as tc:
        with (
            tc.tile_pool(name="const", bufs=1) as constp,
            tc.tile_pool(name="tab", bufs=1) as tabp,
            tc.tile_pool(name="acc", bufs=1) as accp,
            tc.tile_pool(name="msg", bufs=2) as msgp,
            tc.tile_pool(name="idx", bufs=2) as idxp,
            tc.tile_pool(name="sch", bufs=2) as schp,
            tc.tile_pool(name="mt", bufs=3) as mtp,
            tc.tile_pool(name="oh", bufs=4) as ohp,
            tc.tile_pool(name="stage", bufs=3) as stp,
            tc.tile_pool(name="pst", bufs=2, space="PSUM") as pstp,
            tc.tile_pool(name="psa", bufs=2, space="PSUM") as psap,
            tc.tile_pool(name="psh", bufs=2, space="PSUM") as pshp,
        ):
            w1_s = constp.tile([cfg.din, dh], f32)
            nc.sync.dma_start(w1_s[:], W1p[:])
            w2_s = constp.tile([dh, dout], f32)
            nc.sync.dma_start(w2_s[:], W2p[:])
            b1_s = constp.tile([P, dh], f32)
            nc.sync.dma_start(b1_s[:], b1p[:1, :].to_broadcast((P, dh)))
            b2_s = constp.tile([P, dout], f32)
            nc.sync.dma_start(b2_s[:], b2p[:1, :].to_broadcast((P, dout)))
            d2_s = constp.tile([P, ntile], f32)
            nc.sync.dma_start(d2_s[:], d2p[:])
            iota_s = constp.tile([P, P], f32)
            nc.sync.dma_start(iota_s[:], iotap[:])
            sel_s = constp.tile([dh, dh], f32)      # [:, 0:32]=even, 32:64=odd
            nc.sync.dma_start(sel_s[:], selp[:])
            ident = constp.tile([P, P], f32)
            make_identity(nc, ident[:])

            acc = accp.tile([P, ntile, dh], f32)

            for _rep in range(reps):
                # ---- phase A: h1T chunks -> sel matmuls -> inter table
                inter = tabp.tile([32, TC, 2], f32, tag="tab")
                nc.vector.memset(inter[:, shard:, :], 0.0)
                NCH = 512
                nchk = (shard + NCH - 1) // NCH
                for j in range(nchk):
                    nj = min(NCH, shard - j * NCH)
                    xc = stp.tile([cfg.din, NCH], f32, tag="xc")
                    nc.sync.dma_start(xc[:, :nj],
                                      xT[:, j * NCH:j * NCH + nj])
                    ph = pshp.tile([dh, NCH], f32, tag="ph")
                    nc.tensor.matmul(ph[:, :nj], lhsT=w1_s[:], rhs=xc[:, :nj],
                                     start=True, stop=True)
                    hc = stp.tile([dh, NCH], f32, tag="hc")
                    nc.scalar.copy(hc[:, :nj], ph[:, :nj])
                    pe = pshp.tile([dh, NCH], f32, tag="ph")
                    nc.tensor.matmul(pe[:32, :nj], lhsT=sel_s[:, 0:32],
                                     rhs=hc[:, :nj], start=True, stop=True)
                    nc.vector.tensor_copy(
                        inter[:, j * NCH:j * NCH + nj, 0], pe[:32, :nj])
                    po = pshp.tile([dh, NCH], f32, tag="ph")
                    nc.tensor.matmul(po[:32, :nj], lhsT=sel_s[:, 32:64],
                                     rhs=hc[:, :nj], start=True, stop=True)
                    nc.vector.tensor_copy(
                        inter[:, j * NCH:j * NCH + nj, 1], po[:32, :nj])
                nc.sync.dma_start(ag_in[0][:, :],
                                  inter[:].rearrange("p a b -> p (a b)"))
                # prefill acc1 = dinv^2 * h1 (natural)
                for t in range(ntile):
                    nj = min(128, shard - t * 128)
                    xc = stp.tile([cfg.din, 128], f32, tag="xn")
                    nc.sync.dma_start(xc[:, :nj],
                                      xT[:, t * 128:t * 128 + nj])
                    ph = psap.tile([P, dh], f32, tag="pa")
                    nc.tensor.matmul(ph[:nj, :], lhsT=xc[:, :nj],
                                     rhs=w1_s[:], start=True, stop=True)
                    if nj < 128:
                        nc.vector.memset(acc[:, t, :], 0.0)
                    nc.vector.tensor_scalar(acc[:nj, t, :], ph[:nj, :],
                                            d2_s[:nj, t:t + 1], None,
                                            mybir.AluOpType.mult)

                def allgather(l):
                    nc.gpsimd.collective_compute(
                        "AllGather", mybir.AluOpType.bypass,
                        replica_groups=[core_ids],
                        ins=[ag_in[l][:].opt()],
                        outs=[tabd[l][:].opt()],
                    )

                def do_layer(l):
                    for r, sc in enumerate(scheds):
                        tab = tabp.tile([P, TC, 2], f32, tag="tab")
                        for st in range(cfg.nstream):
                            w = 2 * r + st // 2
                            nc.sync.dma_start(
                                tab[32 * st:32 * st + 32, :, :],
                                tabd[l][32 * w:32 * w + 32, :].rearrange(
                                    "p (a b) -> p a b", b=2))
                        gidx_s = idxp.tile([P, sc.S // 16], i16, tag="gidx")
                        nc.sync.dma_start(gidx_s[:], gidxp[r][:])
                        ncr = int(sc.nchunk.sum())
                        dc_s = schp.tile([P, ncr], f32, tag="dc")
                        nc.sync.dma_start(dc_s[:], dcp[r][:])
                        wg_s = schp.tile([P, ncr], f32, tag="wg")
                        nc.sync.dma_start(wg_s[:], wgp[r][:])

                        cur = [None] * cfg.nstream
                        ncmax = int(sc.nchunk.max())
                        for sb in range(sc.nsub):
                            ns = min(SUB, sc.Sused - sb * SUB)
                            msg = msgp.tile([P, SUB, 2], f32, tag="msg")
                            nc.gpsimd.ap_gather(
                                msg[:, :ns, :], tab[:],
                                gidx_s[:, sb * SUB // 16:(sb * SUB + ns) // 16],
                                channels=P, num_elems=TC, d=2, num_idxs=ns)
                            for jj in range(ns // CH):
                                j = sb * (SUB // CH) + jj
                                if j >= ncmax:
                                    break
                                # one shared transpose for all 4 streams
                                seg = msg[:, jj * CH:(jj + 1) * CH, :]
                                ptA = pstp.tile([P, P], f32, tag="trA")
                                nc.tensor.transpose(
                                    ptA[:], seg[:, :, 0], ident[:])
                                ptB = pstp.tile([P, P], f32, tag="trB")
                                nc.tensor.transpose(
                                    ptB[:], seg[:, :, 1], ident[:])
                                mt = mtp.tile([P, P, 2], f32, tag="mt")
                                nc.scalar.copy(mt[:, :, 0], ptA[:])
                                nc.scalar.copy(mt[:, :, 1], ptB[:])
                                for st in range(cfg.nstream):
                                    if j >= sc.nchunk[st]:
                                        continue
                                    o = sc.coff[st]
                                    t = int(sc.chunk_tile[st][j])
                                    first = bool(sc.chunk_first[st][j])
                                    last = bool(sc.chunk_last[st][j])
                                    oh = ohp.tile([P, P], f32, tag="oh")
                                    nc.vector.tensor_scalar(
                                        oh[:], iota_s[:],
                                        dc_s[:, o + j:o + j + 1],
                                        wg_s[:, o + j:o + j + 1],
                                        mybir.AluOpType.is_equal,
                                        mybir.AluOpType.mult)
                                    pa = psap.tile([P, dh], f32, tag="pa")
                                    nc.tensor.matmul(
                                        pa[:], lhsT=oh[:],
                                        rhs=mt[:, 32 * st:32 * st + 32, :]
                                        .rearrange("p a b -> p (a b)"),
                                        start=True, stop=True)
                                    nc.vector.tensor_add(
                                        acc[:, t, :], acc[:, t, :], pa[:])

                allgather(0)
                do_layer(0)

                # ---- layer boundary: hid = lrelu(acc+b1) -> inter; prefill
                inter = tabp.tile([32, TC, 2], f32, tag="tab")
                nc.vector.memset(inter[:, shard:, :], 0.0)
                for t in range(ntile):
                    nj = min(128, shard - t * 128)
                    v = acc[:, t, :]
                    nc.vector.tensor_add(v, v, b1_s[:])
                    t1 = stp.tile([P, dh], f32, tag="lr")
                    nc.vector.tensor_scalar(t1[:], v, 0.01, None,
                                            mybir.AluOpType.mult)
                    nc.vector.tensor_max(v, v, t1[:])
                    pt = pstp.tile([64, P], f32, tag="trA")
                    nc.tensor.transpose(pt[:], v, ident[:])
                    hc = stp.tile([dh, P], f32, tag="hidT")
                    nc.scalar.copy(hc[:], pt[:])
                    pe = pshp.tile([dh, NCH], f32, tag="ph")
                    nc.tensor.matmul(pe[:32, :128], lhsT=sel_s[:, 0:32],
                                     rhs=hc[:], start=True, stop=True)
                    nc.vector.tensor_copy(
                        inter[:, t * 128:t * 128 + nj, 0], pe[:32, :nj])
                    po = pshp.tile([dh, NCH], f32, tag="ph")
                    nc.tensor.matmul(po[:32, :128], lhsT=sel_s[:, 32:64],
                                     rhs=hc[:], start=True, stop=True)
                    nc.vector.tensor_copy(
                        inter[:, t * 128:t * 128 + nj, 1], po[:32, :nj])
                    nc.vector.tensor_scalar(v, v, d2_s[:, t:t + 1], None,
                                            mybir.AluOpType.mult)
                nc.sync.dma_start(ag_in[1][:, :],
                                  inter[:].rearrange("p a b -> p (a b)"))

                allgather(1)
                do_layer(1)

                # ---- out = acc2 @ W2 + b2
                for t in range(ntile):
                    nj = min(128, shard - t * 128)
                    pt = pstp.tile([64, P], f32, tag="trA")
                    nc.tensor.transpose(pt[:], acc[:, t, :], ident[:])
                    a2 = stp.tile([dh, P], f32, tag="a2T")
                    nc.scalar.copy(a2[:], pt[:])
                    po = psap.tile([P, dh], f32, tag="pa")
                    nc.tensor.matmul(po[:, :dout], lhsT=a2[:], rhs=w2_s[:],
                                     start=True, stop=True)
                    oc = stp.tile([P, dout], f32, tag="oc")
                    nc.vector.tensor_add(oc[:], po[:, :dout], b2_s[:])
                    nc.sync.dma_start(outp[t * 128:t * 128 + nj, :],
                                      oc[:nj, :])

    nc.compile()
    return nc


# ---------------------------------------------------------------- inputs

def make_in_maps(inputs, scheds, dinv, cfg: Cfg):
    x = np.ascontiguousarray(np.asarray(inputs["x"], np.float32))
    W1 = np.ascontiguousarray(np.asarray(inputs["W1"], np.float32))
    b1 = np.asarray(inputs["b1"], np.float32).reshape(1, -1)
    W2 = np.ascontiguousarray(np.asarray(inputs["W2"], np.float32))
    b2 = np.asarray(inputs["b2"], np.float32).reshape(1, -1)
    iota = np.ascontiguousarray(
        np.broadcast_to(np.arange(128, dtype=np.float32), (128, 128)))
    sel = np.zeros((64, 64), np.float32)
    for p in range(32):
        sel[2 * p, p] = 1.0          # even cols
        sel[2 * p + 1, 32 + p] = 1.0  # odd cols

    in_maps = []
    for c in range(cfg.ncores):
        sl = slice(c * cfg.shard, (c + 1) * cfg.shard)
        xTc = np.ascontiguousarray(x[sl].T)
        d2 = np.zeros(cfg.ntile * 128, np.float32)
        d2[:cfg.shard] = (dinv[sl].astype(np.float64) ** 2).astype(np.float32)
        d2col = np.ascontiguousarray(d2.reshape(cfg.ntile, 128).T)
        m = {"xT": xTc, "W1": W1, "b1": np.ascontiguousarray(b1),
             "W2": W2, "b2": np.ascontiguousarray(b2),
             "dinv2": d2col, "iota": iota, "sel": sel}
        for r in range(4):
            m[f"gidx{r}"] = scheds[r].gidx[c]
            m[f"dstcol{r}"] = scheds[r].dstcol[c]
            m[f"wgt{r}"] = scheds[r].wgt[c]
        in_maps.append(m)
    return in_maps


# ---------------------------------------------------------------- entry

def kernel(**inputs):
    from concourse.bass_utils import run_bass_kernel_spmd
    cfg = CFG
    scheds, dinv = build_sched(inputs["edge_index"], cfg)
    nc = build_bass(scheds, cfg)
    in_maps = make_in_maps(inputs, scheds, dinv, cfg)
    core_ids = list(range(cfg.ncores))
    res = run_bass_kernel_spmd(nc, in_maps, core_ids).results
    out = np.concatenate([res[c]["out"][:cfg.shard] for c in core_ids], axis=0)
    return out.astype(np.float32)
